# revision 1
# baseline (speedup 1.0000x reference)
"""MSE-style custom loss on 8 Trainium2 NeuronCores.

reference: d = |input - target|; conditional 0.8 scale of d[0] when
d[0] in {3,4,5,6}; return mean(d*d).

Strategy (data-parallel, memory-bound):
  - Split the 32M-element 1-D tensors into 8 contiguous shards (4M each).
  - Per core: stream [128 x F] fp32 tiles of both operands from DRAM,
    d = a - b on the vector engine, then Square activation on the scalar
    engine with accum_out -> per-partition partial sums (one column per
    tile).  2 compute ops per element, both engines well under the DMA
    roofline (~32 MiB/core @ ~358 GB/s ~ 94 us).
  - Host: sum the 8 x [128 x TILES] partials in f64, apply the d[0]
    fixup (only touches one element), divide by N.
"""

import numpy as np

N = 33554432
N_CORES = 8
SHARD = N // N_CORES          # 4194304
P = 128
F = 2048                      # tile free dim  -> 1 MiB fp32 tiles
TILES = SHARD // (P * F)      # 16
DMA_BUFS = 4

_cache = {}


def _get_program():
    if "nc" in _cache:
        return _cache["nc"]

    import concourse.bass as bass  # noqa: F401  (registers engine types)
    import concourse.tile as tile
    from concourse import bacc, mybir

    nc = bacc.Bacc("TRN2", target_bir_lowering=False, debug=False)
    a_d = nc.dram_tensor("input", [TILES, P, F], mybir.dt.float32,
                         kind="ExternalInput").ap()
    b_d = nc.dram_tensor("target", [TILES, P, F], mybir.dt.float32,
                         kind="ExternalInput").ap()
    out_d = nc.dram_tensor("partial", [P, TILES], mybir.dt.float32,
                           kind="ExternalOutput").ap()

    with tile.TileContext(nc) as tc:
        with tc.tile_pool(name="a", bufs=DMA_BUFS) as pa, \
             tc.tile_pool(name="b", bufs=DMA_BUFS) as pb, \
             tc.tile_pool(name="acc", bufs=1) as pacc:
            acc = pacc.tile([P, TILES], mybir.dt.float32)
            for i in range(TILES):
                ta = pa.tile([P, F], mybir.dt.float32)
                nc.sync.dma_start(ta[:], a_d[i])
                tb = pb.tile([P, F], mybir.dt.float32)
                nc.sync.dma_start(tb[:], b_d[i])
                nc.vector.tensor_sub(ta[:], ta[:], tb[:])
                nc.scalar.activation(ta[:], ta[:],
                                     mybir.ActivationFunctionType.Square,
                                     accum_out=acc[:, i:i + 1])
            nc.sync.dma_start(out_d[:], acc[:])

    nc.compile()
    _cache["nc"] = nc
    return nc


def run_spmd(input, target, trace=False, **kw):
    """Run the sharded kernel; returns (partial_sums_f64, BassKernelResults)."""
    from concourse.bass_utils import run_bass_kernel_spmd

    nc = _get_program()
    a = np.ascontiguousarray(np.asarray(input, dtype=np.float32)
                             ).reshape(N_CORES, TILES, P, F)
    b = np.ascontiguousarray(np.asarray(target, dtype=np.float32)
                             ).reshape(N_CORES, TILES, P, F)
    in_maps = [{"input": a[c], "target": b[c]} for c in range(N_CORES)]
    br = run_bass_kernel_spmd(nc, in_maps, list(range(N_CORES)),
                              trace=trace, **kw)
    total = 0.0
    for r in br.results:
        total += float(np.sum(r["partial"], dtype=np.float64))
    return total, br


def kernel(input, target):
    input = np.asarray(input)
    target = np.asarray(target)
    total, _ = run_spmd(input, target)

    # res[0] fixup, faithful to the fp32 reference semantics.
    d0 = np.float32(abs(np.float32(input.reshape(-1)[0]) -
                        np.float32(target.reshape(-1)[0])))
    if d0 in (np.float32(3.0), np.float32(4.0),
              np.float32(5.0), np.float32(6.0)):
        d0f = np.float32(d0 * np.float32(0.8))
        total += float(d0f) * float(d0f) - float(d0) * float(d0)

    return np.array(total / N, dtype=np.float32)
